# revision 1
# baseline (speedup 1.0000x reference)
"""Trainium2 Bass kernel for nn_AGCRNCellWithMLP (AGCRN cell with per-node MLP weights).

Math (with nodes_ind == arange(N), which the harness guarantees):
    xh       = concat([x, h], -1)                      # [N, 129]
    combined = adj @ xh                                # [N, 129]
    r = sigmoid(mlp(combined, q, W_r, b_r))            # [N, 64]
    u = sigmoid(mlp(combined, q, W_u, b_u))
    h2 = r * h
    cand = tanh(mlp(concat([x, h2], -1), q, W_c, b_c))
    out = (1 - u) * h2 + u * cand
where mlp(v, q, W, b)[n, o] = sum_{d,i} q[n,d] v[n,i] W[d,i,o] + (q @ b)[n, o].

Sharding: data-parallel over nodes, 512 rows per core x 8 cores, fully
independent per core (no collectives); host replicates x/h and pre-transposes
per-core slices. All matmul tensors are float32r (fp32 storage, fast PE mode;
needs moving dim >= 256), so every matmul here uses N=256 node-columns.

Per-core pipeline, run twice (once per 256-node half, pipelined):
  1. combT[129, 256] = (adj_slice @ xh)^T directly: lhsT = xh k-tiles
     ([128,128] + [128,1] column splits), rhs = adjT k-slabs [128, 256].
     The 129th feature row accumulates in spare columns of the same PSUM bank.
  2. Z^T[r=(i*16+d), n] = V[i, n] * qT[d, n] for V in {combT, xh2T}, built 128
     rows per tile: a = sel_t.T @ V (PE replication matmul), z = a * q_rep (DVE).
  3. Gate preact G^T[64, 256] = b_g^T qT + sum_t Wf_g[t]^T Z^T[t] in PSUM;
     sigmoid/tanh on ScalarE; h2/output elementwise on DVE; DMA out f32.
"""
import sys

sys.path.insert(0, "/opt/trn_rl_repo")

import numpy as np

N = 4096
IN = 64
QD = 16
CI = 2 * IN + 1          # 129
NCORES = 8
NS = N // NCORES         # 512 nodes per core
NH = NS // 2             # 256 nodes per half
KT = N // 128            # 32 k-tiles for the adj matmul
RT = 17                  # ceil(CI*QD/128) z-tiles (2176 padded rows)
RPAD = RT * 128          # 2176
CI2 = CI + 1             # xh slab width: 129 + duplicated last col

_CACHE = {}


def build_nc():
    import concourse.bass as bass
    import concourse.bacc as bacc
    import concourse.tile as tile
    import concourse.mybir as mybir

    F32 = mybir.dt.float32
    F32R = mybir.dt.float32r
    ACT = mybir.ActivationFunctionType

    nc = bacc.Bacc()
    dp = nc.declare_dram_parameter
    adjT_e = dp("adjT", [128, 2 * KT * NH], F32R, isOutput=False)  # SBUF-layout [p, (half,ktile,n)]
    xh_e = dp("xh", [128, KT * CI2], F32R, isOutput=False)    # SBUF-layout [p, (ktile,f)]
    qT_e = dp("qT", [QD, NS], F32R, isOutput=False)
    qrep_e = dp("qrep", [128, 2 * NS], F32, isOutput=False)   # qT tiled x8 down partitions, x2 along free
    xT_e = dp("xT", [IN + 2, NS], F32R, isOutput=False)
    hT_e = dp("hT", [IN, NS], F32, isOutput=False)
    wfru_e = dp("wfru", [128, RT * 2 * IN], F32R, isOutput=False)
    wfc_e = dp("wfc", [128, RT * IN], F32R, isOutput=False)
    bru_e = dp("bru", [QD, 2 * IN], F32R, isOutput=False)
    bc_e = dp("bc", [QD, IN], F32R, isOutput=False)
    sel_e = dp("sel", [128, 16 * 128], F32R, isOutput=False)
    sel16_e = dp("sel16", [2, 128], F32R, isOutput=False)
    out_e = dp("out", [IN, NS], F32, isOutput=True)

    with tile.TileContext(nc) as tc:
        with tc.tile_pool(name="const", bufs=1) as cpool, \
             tc.tile_pool(name="big", bufs=1) as bigpool, \
             tc.tile_pool(name="half", bufs=2) as hpool, \
             tc.tile_pool(name="zt", bufs=6) as ztpool, \
             tc.tile_pool(name="psC", bufs=1, space="PSUM") as psC, \
             tc.tile_pool(name="psZ", bufs=3, space="PSUM") as psZ, \
             tc.tile_pool(name="psG", bufs=2, space="PSUM") as psG:

            # ---- static loads (emission order ~= DMA service order) -----------
            # xh first (first matmul needs it), then adjT half 0, then the
            # small gate constants, then adjT half 1.
            xh = bigpool.tile([128, KT * CI2], F32R)
            adjT = bigpool.tile([128, 2 * KT * NH], F32R)

            def load_adjT_chunk(h, ch, nch=4):
                w = KT // nch
                lo = (h * KT + ch * w) * NH
                hi = (h * KT + (ch + 1) * w) * NH
                nc.sync.dma_start(adjT[:, lo:hi], adjT_e[:, lo:hi])

            # adjT half-0 in ramped chunks so the first matmul starts ASAP;
            # gate constants ordered by first-use time; adjT half-1 interleaved
            # so h1 adj matmuls can fill PE gaps during h0's gate phases.
            adj_off = [0]

            def load_adjT_slabs(h, nslab):
                lo = (h * KT + adj_off[0]) * NH
                hi = (h * KT + adj_off[0] + nslab) * NH
                nc.sync.dma_start(adjT[:, lo:hi], adjT_e[:, lo:hi])
                adj_off[0] = (adj_off[0] + nslab) % KT

            xh_off = [0]

            def load_xh_slabs(nslab):
                xlo = xh_off[0] * CI2
                xhi = (xh_off[0] + nslab) * CI2
                nc.sync.dma_start(xh[:, xlo:xhi], xh_e[:, xlo:xhi])
                xh_off[0] += nslab

            for ch in range(8):
                load_xh_slabs(4)
                load_adjT_slabs(0, 4)
            wfru = cpool.tile([128, RT * 2 * IN], F32R, tag="wfru")
            nc.sync.dma_start(wfru[:], wfru_e[:])
            wfc = cpool.tile([128, RT * IN], F32R, tag="wfc")
            nc.sync.dma_start(wfc[:], wfc_e[:])
            bru = cpool.tile([QD, 2 * IN], F32R, tag="bru")
            nc.sync.dma_start(bru[:], bru_e[:])
            bc = cpool.tile([QD, IN], F32R, tag="bc")
            nc.sync.dma_start(bc[:], bc_e[:])
            sel = cpool.tile([128, 16 * 128], F32R, tag="sel")
            nc.sync.dma_start(sel[:], sel_e[:])
            sel16 = cpool.tile([2, 128], F32R, tag="sel16")
            nc.sync.dma_start(sel16[:], sel16_e[:])
            qT = cpool.tile([QD, NS], F32R, tag="qT")
            nc.sync.dma_start(qT[:], qT_e[:])
            qrep = cpool.tile([128, 2, NS], F32, tag="qrep")
            nc.sync.dma_start(qrep[:], qrep_e[:])
            hT = cpool.tile([IN, NS], F32, tag="hT")
            nc.sync.dma_start(hT[:], hT_e[:])
            xT = cpool.tile([IN + 2, NS], F32R, tag="xT")
            nc.sync.dma_start(xT[:], xT_e[:])
            for _ in range(4):
                load_adjT_slabs(1, 8)

            def gate_pass(h, Vm, Vt, wft, bt, ps_g, mw, tag):
                cols = slice(h * NH, (h + 1) * NH)
                nc.tensor.matmul(ps_g[:], bt[:], qT[:, cols],
                                 start=True, stop=False)
                for t in range(RT):
                    az = psZ.tile([128, NH], F32, tag="az", name=f"az{tag}{h}_{t}")
                    if t < 16:
                        nc.tensor.matmul(az[:], sel[:, t * 128:(t + 1) * 128],
                                         Vm[:], start=True, stop=True)
                    else:
                        nc.tensor.matmul(az[:], sel16[:], Vt[:],
                                         start=True, stop=True)
                    z = ztpool.tile([128, NH], F32R, tag="z")
                    nc.vector.tensor_mul(z[:], az[:], qrep[:, 0, cols])
                    nc.tensor.matmul(ps_g[:], wft[:, t * mw:(t + 1) * mw], z[:],
                                     start=False, stop=(t == RT - 1))

            for h in range(2):
                cols = slice(h * NH, (h + 1) * NH)

                pc = psC.tile([128, NH], F32, tag="pc", name=f"pc{h}")
                pl = psC.tile([2, NH], F32, tag="pl", name=f"pl{h}")
                for t in range(KT):
                    rhs = adjT[:, (h * KT + t) * NH:(h * KT + t + 1) * NH]
                    nc.tensor.matmul(pc[:], xh[:, t * CI2: t * CI2 + 128], rhs,
                                     start=(t == 0), stop=(t == KT - 1))
                    nc.tensor.matmul(pl[:], xh[:, t * CI2 + 128: t * CI2 + 130], rhs,
                                     start=(t == 0), stop=(t == KT - 1))
                combT = hpool.tile([128, NH], F32R, tag="combT", name=f"combT{h}")
                nc.vector.tensor_copy(combT[:], pc[:])
                combTt = hpool.tile([2, NH], F32R, tag="combTt", name=f"combTt{h}")
                nc.vector.tensor_copy(combTt[:], pl[:])

                ps_ru = psG.tile([2 * IN, NH], F32, tag="g", name=f"gru{h}")
                gate_pass(h, combT, combTt, wfru, bru, ps_ru, 2 * IN, "ru")
                r_sb = hpool.tile([IN, NH], F32, tag="r_sb", name=f"r{h}")
                nc.scalar.activation(r_sb[:], ps_ru[0:IN, :], ACT.Sigmoid)
                u_sb = hpool.tile([IN, NH], F32, tag="u_sb", name=f"u{h}")
                nc.scalar.activation(u_sb[:], ps_ru[IN:2 * IN, :], ACT.Sigmoid)

                xh2T = hpool.tile([128, NH], F32R, tag="xh2T", name=f"xh2T{h}")
                xh2Tt = hpool.tile([2, NH], F32R, tag="xh2Tt", name=f"xh2Tt{h}")
                nc.vector.tensor_mul(xh2T[0:IN, :], r_sb[:], hT[:, cols])
                nc.vector.tensor_copy(xh2T[IN:128, :], xT[0:IN, cols])
                nc.vector.tensor_copy(xh2Tt[:], xT[IN:IN + 2, cols])

                ps_c2 = psG.tile([IN, NH], F32, tag="g", name=f"gc{h}")
                gate_pass(h, xh2T, xh2Tt, wfc, bc, ps_c2, IN, "c")
                cand = hpool.tile([IN, NH], F32, tag="cand", name=f"cand{h}")
                nc.scalar.activation(cand[:], ps_c2[:], ACT.Tanh)

                dt_ = hpool.tile([IN, NH], F32, tag="dt", name=f"dt{h}")
                nc.vector.tensor_sub(dt_[:], cand[:], xh2T[0:IN, :])
                et = hpool.tile([IN, NH], F32, tag="et", name=f"et{h}")
                nc.vector.tensor_mul(et[:], u_sb[:], dt_[:])
                outT = hpool.tile([IN, NH], F32, tag="outT", name=f"outT{h}")
                nc.vector.tensor_add(outT[:], xh2T[0:IN, :], et[:])
                nc.sync.dma_start(out_e[:, cols], outT[:])
    nc.compile()
    return nc


def _f32(a):
    return np.ascontiguousarray(np.asarray(a, np.float32))


def prep_in_maps(x, h, query_vectors, adj, nodes_ind, W_r, b_r, W_u, b_u, W_c, b_c):
    x = _f32(x)
    h = _f32(h)
    q = _f32(query_vectors)
    adj = np.asarray(adj, np.float32)
    ni = np.asarray(nodes_ind)
    assert np.array_equal(ni, np.arange(N)), "kernel assumes nodes_ind == arange(N)"

    xh = np.concatenate([x, h, h[:, -1:]], axis=-1)           # [N, 130] (last col 2x)
    xh_sb = _f32(xh.reshape(KT, 128, CI2).transpose(1, 0, 2).reshape(128, KT * CI2))
    wfs, bs = {}, {}
    # xh2 feature order for gate c: [h2(0..63), x(0..63), x(64)]
    perm_c = list(range(IN + 1, CI)) + list(range(0, IN)) + [IN]
    for g, W, b in (("r", W_r, b_r), ("u", W_u, b_u), ("c", W_c, b_c)):
        Wt = np.asarray(W, np.float32).transpose(1, 0, 2)     # [129(i), 16, 64]
        if g == "c":
            Wt = Wt[perm_c]
        Wim = Wt.reshape(CI * QD, IN)
        Wp = np.concatenate(
            [Wim, np.zeros((RPAD - CI * QD, IN), np.float32)], axis=0)
        wfs[g] = Wp.reshape(RT, 128, IN)
        bs[g] = np.asarray(b, np.float32)

    wfru = _f32(np.concatenate([wfs["r"], wfs["u"]], axis=2)
                .transpose(1, 0, 2).reshape(128, RT * 2 * IN))
    wfc = _f32(wfs["c"].transpose(1, 0, 2).reshape(128, RT * IN))
    bru = _f32(np.concatenate([bs["r"], bs["u"]], axis=1))
    bc = _f32(bs["c"])

    sel = np.zeros((128, 16 * 128), np.float32)
    for t in range(16):
        for p in range(128):
            sel[8 * t + p // 16, t * 128 + p] = 1.0
    sel16 = np.zeros((2, 128), np.float32)
    sel16[0, 0:16] = 1.0

    in_maps = []
    for c in range(NCORES):
        s = slice(c * NS, (c + 1) * NS)
        qTc = _f32(q[s].T)                                    # [16, 512]
        # adjT SBUF layout [p, (half, ktile, n)]: adjT[k, n] = adj[s][n, k]
        a = adj[s].T.reshape(KT, 128, 2, NH)                  # [t, p, half, n]
        adjT_sb = _f32(a.transpose(1, 2, 0, 3).reshape(128, 2 * KT * NH))
        in_maps.append({
            "adjT": adjT_sb,
            "xh": xh_sb,
            "qT": qTc,
            "qrep": np.ascontiguousarray(np.tile(qTc, (2 * (128 // QD), 1)).reshape(2, 128, NS).transpose(1, 0, 2).reshape(128, 2 * NS)),
            "xT": _f32(np.concatenate([x[s].T, np.zeros((1, NS), np.float32)], axis=0)),
            "hT": _f32(h[s].T),
            "wfru": wfru, "wfc": wfc, "bru": bru, "bc": bc,
            "sel": sel, "sel16": sel16,
        })
    return in_maps


def kernel(**inputs):
    from concourse.bass_utils import run_bass_kernel_spmd

    if "nc" not in _CACHE:
        _CACHE["nc"] = build_nc()
    nc = _CACHE["nc"]
    in_maps = prep_in_maps(**inputs)
    res = run_bass_kernel_spmd(nc, in_maps, core_ids=list(range(NCORES)))
    out = np.empty((N, IN), np.float32)
    for c in range(NCORES):
        out[c * NS:(c + 1) * NS, :] = res.results[c]["out"].T
    return out



# revision 6
# speedup vs baseline: 1.0115x; 1.0115x over previous
"""Trainium2 Bass kernel for nn_AGCRNCellWithMLP (AGCRN cell with per-node MLP weights).

Math (with nodes_ind == arange(N), which the harness guarantees):
    xh       = concat([x, h], -1)                      # [N, 129]
    combined = adj @ xh                                # [N, 129]
    r = sigmoid(mlp(combined, q, W_r, b_r))            # [N, 64]
    u = sigmoid(mlp(combined, q, W_u, b_u))
    h2 = r * h
    cand = tanh(mlp(concat([x, h2], -1), q, W_c, b_c))
    out = (1 - u) * h2 + u * cand
where mlp(v, q, W, b)[n, o] = sum_{d,i} q[n,d] v[n,i] W[d,i,o] + (q @ b)[n, o].

Sharding: data-parallel over nodes, 512 rows per core x 8 cores, fully
independent per core (no collectives). All matmul operands are fp16 (verified
rel err ~3e-3 vs the 2e-2 gate); PSUM/final output fp32.

Per-core pipeline, 2 node-halves of 256, pipelined:
  1. combT[129, 256] = (adj_slice @ xh)^T: lhsT = xh k-tiles [128,128], rhs =
     adjT k-slabs [128, 256] -> pc PSUM. Feature 128 via 4-way column-tiled
     1-row matmuls (4 k-tiles concurrent, partials at psum rows 0/32/64/96),
     reduced by the sel17 matmul when building the 17th z-tile.
  2. z[(i*16+d), n] = V[i,n]*q[d,n] built 128 rows/tile: az = sel_t.T @ V (PE
     replication matmul) -> ScalarE copy PSUM->SBUF fp16 -> DVE mul by qrep
     (fp16 2x mode).
  3. Gate preact G^T = b_g^T qT + sum_t Wf_g[t]^T z_t in PSUM; sigmoid/tanh on
     ScalarE; output elementwise on DVE.
  4. Gate c's x-part z-tiles (xq = x (x) q, input-only) come pre-multiplied
     from the host; their Wf matmuls run during the DMA-bound adj phase.
"""
import sys

sys.path.insert(0, "/opt/trn_rl_repo")

import numpy as np

N = 4096
IN = 64
QD = 16
CI = 2 * IN + 1          # 129
NCORES = 8
NS = N // NCORES         # 512 nodes per core
NH = NS // 2             # 256 nodes per half
KT = N // 128            # 32 k-tiles for the adj matmul
RT = 17                  # z tiles per gate input (2176 padded rows)
XQT = 9                  # xq tiles (c-gate x-part: 65*16=1040 rows -> 9 tiles)
CHT = 8                  # c-gate h-part z tiles (64*16=1024 rows)
CI2 = CI + 1             # xh slab width: 129 + pad col

_CACHE = {}


def build_nc():
    import concourse.bass as bass
    import concourse.bacc as bacc
    import concourse.tile as tile
    import concourse.mybir as mybir

    F32 = mybir.dt.float32
    F16 = mybir.dt.float16
    ACT = mybir.ActivationFunctionType

    nc = bacc.Bacc()
    dp = nc.declare_dram_parameter
    adjT_e = dp("adjT", [128, 2 * KT * NH], F16, isOutput=False)  # [p,(half,kt,n)]
    xh_e = dp("xh", [128, KT * CI2], F16, isOutput=False)     # [p,(kt,f)]
    xq_e = dp("xq", [128, XQT * NS], F16, isOutput=False)     # c-gate x-part z
    qT_e = dp("qT", [QD, NS], F16, isOutput=False)
    qrep_e = dp("qrep", [128, NS], F16, isOutput=False)       # q[p%16, n]
    hT_e = dp("hT", [IN, NS], F16, isOutput=False)
    wfru_e = dp("wfru", [128, RT * 2 * IN], F16, isOutput=False)
    wfc_e = dp("wfc", [128, RT * IN], F16, isOutput=False)
    bru_e = dp("bru", [QD, 2 * IN], F16, isOutput=False)
    bc_e = dp("bc", [QD, IN], F16, isOutput=False)
    sel_e = dp("sel", [128, 16 * 128], F16, isOutput=False)
    sel17_e = dp("sel17", [128, 128], F16, isOutput=False)    # pl-partial reduce
    out_e = dp("out", [IN, NS], F32, isOutput=True)

    with tile.TileContext(nc) as tc:
        with tc.tile_pool(name="const", bufs=1) as cpool, \
             tc.tile_pool(name="big", bufs=1) as bigpool, \
             tc.tile_pool(name="half", bufs=2) as hpool, \
             tc.tile_pool(name="azs", bufs=4) as azpool, \
             tc.tile_pool(name="zt", bufs=6) as ztpool, \
             tc.tile_pool(name="psM", bufs=1, space="PSUM") as psM, \
             tc.tile_pool(name="psZ", bufs=3, space="PSUM") as psZ:

            # ---- static loads (emission order ~= DMA service order) -----------
            xh = bigpool.tile([128, KT * CI2], F16)
            adjT = bigpool.tile([128, 2 * KT * NH], F16)
            xq = bigpool.tile([128, XQT * NS], F16)

            adj_off = [0]

            def load_adjT_slabs(h, nslab):
                lo = (h * KT + adj_off[0]) * NH
                hi = (h * KT + adj_off[0] + nslab) * NH
                nc.sync.dma_start(adjT[:, lo:hi], adjT_e[:, lo:hi])
                adj_off[0] = (adj_off[0] + nslab) % KT

            xh_off = [0]

            def load_xh_slabs(nslab):
                xlo = xh_off[0] * CI2
                xhi = (xh_off[0] + nslab) * CI2
                nc.sync.dma_start(xh[:, xlo:xhi], xh_e[:, xlo:xhi])
                xh_off[0] += nslab

            # xh + h0 adj first (ramped), then gate constants by first use,
            # xq midway, then h1 adj.
            for ch in range(8):
                load_xh_slabs(4)
                load_adjT_slabs(0, 4)
            qT = cpool.tile([QD, NS], F16, tag="qT")
            nc.sync.dma_start(qT[:], qT_e[:])
            bru = cpool.tile([QD, 2 * IN], F16, tag="bru")
            nc.sync.dma_start(bru[:], bru_e[:])
            bc = cpool.tile([QD, IN], F16, tag="bc")
            nc.sync.dma_start(bc[:], bc_e[:])
            wfc = cpool.tile([128, RT * IN], F16, tag="wfc")
            nc.sync.dma_start(wfc[:], wfc_e[:])
            for i in range(3):
                nc.sync.dma_start(xq[:, i * 3 * NS:(i + 1) * 3 * NS],
                                  xq_e[:, i * 3 * NS:(i + 1) * 3 * NS])
            sel = cpool.tile([128, 16 * 128], F16, tag="sel")
            nc.sync.dma_start(sel[:], sel_e[:])
            sel17 = cpool.tile([128, 128], F16, tag="sel17")
            nc.sync.dma_start(sel17[:], sel17_e[:])
            qrep = cpool.tile([128, NS], F16, tag="qrep")
            nc.sync.dma_start(qrep[:], qrep_e[:])
            wfru = cpool.tile([128, RT * 2 * IN], F16, tag="wfru")
            nc.sync.dma_start(wfru[:], wfru_e[:])
            hT = cpool.tile([IN, NS], F16, tag="hT")
            nc.sync.dma_start(hT[:], hT_e[:])
            for _ in range(4):
                load_adjT_slabs(1, 8)

            def z_chain(az_ps, cols, ztag, name):
                """PE az already issued -> SE copy fp16 -> DVE mul qrep -> z."""
                az_sb = azpool.tile([128, NH], F16, tag="az_sb", name=f"azs{name}")
                nc.scalar.copy(az_sb[:], az_ps[:])
                z = ztpool.tile([128, NH], F16, tag=ztag, name=f"z{name}")
                nc.vector.tensor_mul(z[:], az_sb[:], qrep[:, cols])
                return z

            # full-width PSUM tiles; each half uses a column slice
            pc_f = psM.tile([128, NS], F32, tag="pc")
            pl_f = psM.tile([128, NS], F32, tag="pl")
            gru_f = psM.tile([2 * IN, NS], F32, tag="gru")
            gc_f = psM.tile([IN, NS], F32, tag="gc")

            for h in range(2):
                cols = slice(h * NH, (h + 1) * NH)

                # -- c-gate PSUM opens early: bias + 9 xq Wf matmuls ---------
                ps_c = gc_f[:, cols]
                nc.tensor.matmul(ps_c, bc[:], qT[:, cols],
                                 start=True, stop=False, skip_group_check=True)
                for t in range(XQT):
                    nc.tensor.matmul(ps_c, wfc[:, t * IN:(t + 1) * IN],
                                     xq[:, t * NS + h * NH: t * NS + h * NH + NH],
                                     start=False, stop=False,
                                     skip_group_check=True)

                # -- adj matmul: main 128 features + col-tiled tail ----------
                pc = pc_f[:, cols]
                pl = pl_f[:, cols]
                for t in range(KT):
                    rhs = adjT[:, (h * KT + t) * NH:(h * KT + t + 1) * NH]
                    nc.tensor.matmul(pc, xh[:, t * CI2: t * CI2 + 128], rhs,
                                     start=(t == 0), stop=(t == KT - 1))
                    j = t % 4
                    nc.tensor.matmul(pl_f[32 * j:32 * j + 1, cols],
                                     xh[:, t * CI2 + 128: t * CI2 + 129], rhs,
                                     start=(t < 4), stop=(t >= KT - 4),
                                     tile_position=(0, 32 * j),
                                     skip_group_check=True)
                combT = hpool.tile([128, NH], F16, tag="combT", name=f"combT{h}")
                nc.scalar.copy(combT[:], pc)
                pl_sb = hpool.tile([128, NH], F16, tag="pl_sb", name=f"pl_sb{h}")
                nc.scalar.copy(pl_sb[:], pl)

                # -- gates r,u: 17 z-tiles (16 from combT + tail via sel17) --
                ps_ru = gru_f[:, cols]
                nc.tensor.matmul(ps_ru, bru[:], qT[:, cols],
                                 start=True, stop=False, skip_group_check=True)
                for t in range(RT):
                    az = psZ.tile([128, NH], F32, tag="az", name=f"azru{h}_{t}")
                    if t < 16:
                        nc.tensor.matmul(az[:], sel[:, t * 128:(t + 1) * 128],
                                         combT[:], start=True, stop=True)
                    else:
                        nc.tensor.matmul(az[:], sel17[:], pl_sb[:],
                                         start=True, stop=True)
                    z = z_chain(az, cols, "zru", f"ru{h}_{t}")
                    nc.tensor.matmul(ps_ru, wfru[:, t * 2 * IN:(t + 1) * 2 * IN],
                                     z[:], start=False, stop=(t == RT - 1),
                                     skip_group_check=True)
                r_sb = hpool.tile([IN, NH], F16, tag="r_sb", name=f"r{h}")
                nc.scalar.activation(r_sb[:], gru_f[0:IN, cols], ACT.Sigmoid)
                u_sb = hpool.tile([IN, NH], F16, tag="u_sb", name=f"u{h}")
                nc.scalar.activation(u_sb[:], gru_f[IN:2 * IN, cols], ACT.Sigmoid)

                h2 = hpool.tile([IN, NH], F16, tag="h2", name=f"h2{h}")
                nc.vector.tensor_mul(h2[:], r_sb[:], hT[:, cols])

                # -- gate c h-part: 8 z-tiles from h2 ------------------------
                for t in range(CHT):
                    az = psZ.tile([128, NH], F32, tag="az", name=f"azc{h}_{t}")
                    nc.tensor.matmul(az[:], sel[0:IN, t * 128:(t + 1) * 128],
                                     h2[:], start=True, stop=True)
                    z = z_chain(az, cols, "zc", f"c{h}_{t}")
                    tt = XQT + t
                    nc.tensor.matmul(ps_c, wfc[:, tt * IN:(tt + 1) * IN],
                                     z[:], start=False, stop=(t == CHT - 1),
                                     skip_group_check=True)
                cand = hpool.tile([IN, NH], F16, tag="cand", name=f"cand{h}")
                nc.scalar.activation(cand[:], ps_c, ACT.Tanh)

                # -- out = h2 + u*(cand - h2) --------------------------------
                dt_ = hpool.tile([IN, NH], F16, tag="dt", name=f"dt{h}")
                nc.vector.tensor_sub(dt_[:], cand[:], h2[:])
                et = hpool.tile([IN, NH], F16, tag="et", name=f"et{h}")
                nc.vector.tensor_mul(et[:], u_sb[:], dt_[:])
                outT = hpool.tile([IN, NH], F32, tag="outT", name=f"outT{h}")
                nc.vector.tensor_add(outT[:], h2[:], et[:])
                nc.sync.dma_start(out_e[:, cols], outT[:])
    nc.compile()
    return nc


def _f16(a):
    return np.ascontiguousarray(np.asarray(a, np.float16))


def prep_in_maps(x, h, query_vectors, adj, nodes_ind, W_r, b_r, W_u, b_u, W_c, b_c):
    x = np.asarray(x, np.float32)
    h = np.asarray(h, np.float32)
    q = np.asarray(q_ := query_vectors, np.float32)
    adj = np.asarray(adj, np.float32)
    ni = np.asarray(nodes_ind)
    assert np.array_equal(ni, np.arange(N)), "kernel assumes nodes_ind == arange(N)"

    xh = np.concatenate([x, h, np.zeros((N, 1), np.float32)], axis=-1)  # [N,130]
    xh_sb = _f16(xh.reshape(KT, 128, CI2).transpose(1, 0, 2).reshape(128, KT * CI2))

    # Wf layouts: z-row r = i*16+d (i-major) for ru; c-gate uses x-part rows
    # (i<65 -> r=i*16+d, 9 tiles padded to 1152) then h-part (i>=65 ->
    # r=1152+(i-65)*16+d, 8 tiles).
    wfs = {}
    for g, W in (("r", W_r), ("u", W_u), ("c", W_c)):
        Wt = np.asarray(W, np.float32).transpose(1, 0, 2)     # [129(i), 16(d), 64]
        if g == "c":
            Wx = Wt[:65].reshape(65 * QD, IN)                 # 1040 rows
            Wx = np.concatenate(
                [Wx, np.zeros((XQT * 128 - 65 * QD, IN), np.float32)], axis=0)
            Wh = Wt[65:].reshape(64 * QD, IN)                 # 1024 rows
            wfs[g] = np.concatenate([Wx, Wh], axis=0).reshape(RT, 128, IN)
        else:
            Wim = Wt.reshape(CI * QD, IN)
            Wp = np.concatenate(
                [Wim, np.zeros((RT * 128 - CI * QD, IN), np.float32)], axis=0)
            wfs[g] = Wp.reshape(RT, 128, IN)

    wfru = _f16(np.concatenate([wfs["r"], wfs["u"]], axis=2)
                .transpose(1, 0, 2).reshape(128, RT * 2 * IN))
    wfc = _f16(wfs["c"].transpose(1, 0, 2).reshape(128, RT * IN))
    bru = _f16(np.concatenate([np.asarray(b_r, np.float32),
                               np.asarray(b_u, np.float32)], axis=1))
    bc = _f16(np.asarray(b_c, np.float32))

    sel = np.zeros((128, 16 * 128), np.float32)
    for t in range(16):
        for p in range(128):
            sel[8 * t + p // 16, t * 128 + p] = 1.0
    # sel17: az17[p<16, n] = sum_j pl_sb[32j, n]  (tail z-tile from 4 partials)
    sel17 = np.zeros((128, 128), np.float32)
    for jj in range(4):
        sel17[32 * jj, 0:16] = 1.0

    in_maps = []
    for c in range(NCORES):
        s = slice(c * NS, (c + 1) * NS)
        qTc = _f16(q[s].T)                                    # [16, 512]
        qrep_c = _f16(np.tile(q[s].T, (128 // QD, 1)))        # [128, 512]
        # adjT SBUF layout [p, (half, ktile, n)]: adjT[k, n] = adj[s][n, k]
        a = adj[s].T.reshape(KT, 128, 2, NH)                  # [t, p, half, n]
        adjT_sb = _f16(a.transpose(1, 2, 0, 3).reshape(128, 2 * KT * NH))
        # xq: c-gate x-part z rows r=i*16+d: xq[r, n] = x[n, i] * q[n, d]
        xq_f = np.einsum("ni,nd->idn", x[s], q[s]).reshape(65 * QD, NS)
        xq_sb = np.zeros((XQT, 128, NS), np.float32)
        xq_sb.reshape(XQT * 128, NS)[:65 * QD] = xq_f
        xq_sb = _f16(xq_sb.transpose(1, 0, 2).reshape(128, XQT * NS))
        in_maps.append({
            "adjT": adjT_sb,
            "xh": xh_sb,
            "xq": xq_sb,
            "qT": qTc,
            "qrep": qrep_c,
            "hT": _f16(h[s].T),
            "wfru": wfru, "wfc": wfc, "bru": bru, "bc": bc,
            "sel": _f16(sel), "sel17": _f16(sel17),
        })
    return in_maps


def kernel(**inputs):
    from concourse.bass_utils import run_bass_kernel_spmd

    if "nc" not in _CACHE:
        _CACHE["nc"] = build_nc()
    nc = _CACHE["nc"]
    in_maps = prep_in_maps(**inputs)
    res = run_bass_kernel_spmd(nc, in_maps, core_ids=list(range(NCORES)))
    out = np.empty((N, IN), np.float32)
    for c in range(NCORES):
        out[c * NS:(c + 1) * NS, :] = res.results[c]["out"].T
    return out


# revision 8
# speedup vs baseline: 1.3902x; 1.3744x over previous
"""Trainium2 Bass kernel for nn_AGCRNCellWithMLP (AGCRN cell with per-node MLP weights).

Math (with nodes_ind == arange(N), which the harness guarantees):
    xh       = concat([x, h], -1)                      # [N, 129]
    combined = adj @ xh                                # [N, 129]
    r = sigmoid(mlp(combined, q, W_r, b_r))            # [N, 64]
    u = sigmoid(mlp(combined, q, W_u, b_u))
    h2 = r * h
    cand = tanh(mlp(concat([x, h2], -1), q, W_c, b_c))
    out = (1 - u) * h2 + u * cand
where mlp(v, q, W, b)[n, o] = sum_{d,i} q[n,d] v[n,i] W[d,i,o] + (q @ b)[n, o].

Sharding: data-parallel over nodes, 512 rows per core x 8 cores, no
collectives. All matmul operands fp16 (rel err ~3e-3 vs 2e-2 gate), PSUM and
final output fp32.

Per-core structure (full 512-node width, d-major gates):
  warmup  qbc_d[128,512] = broadcast of q row d via trivial-weight matmul
          (lhsT=ones[1,128], rhs=qT[d]) -> PSUM -> fp16 SBUF, d=0..15; gate
          bias matmuls open the gru/gc PSUM accumulations. All of this only
          needs tiny DMAs, so it runs while adjT streams in.
  adj     combT[128,512] = (adj_slice @ xh[:, :128])^T: 32 k-tile matmuls,
          lhsT = xh k-tile [128,128], rhs = adjT slab [128,512]. Feature 128
          via 1-col-weight matmuls, 4-way column-tiled (concurrent strips,
          partials at psum rows 0/32/64/96), reduced by the sel17b matmul
          into v128rep[16,512].
  gates   for each d: z_d = V (.) qbc_d on DVE (fp16 2x), then one matmul
          G += Wd^T @ z_d. No replication of V is ever built: 16 z_d tiles
          ARE the z expansion, d-major. Tail feature (i=128) folded as
          s[d,n] = q[d,n]*v128[n] (DVE) + one [16,k] matmul.
          Gate c uses feature order [h2(64) | x(0:64)] so its tail is x[:,64]
          (input-only, host-precomputed x64rep); xh2 needs no extra copies:
          x goes straight into rows 64:128 by DMA, h2 lands in rows 0:64.
"""
import sys

sys.path.insert(0, "/opt/trn_rl_repo")

import numpy as np

N = 4096
IN = 64
QD = 16
CI = 2 * IN + 1          # 129
NCORES = 8
NS = N // NCORES         # 512 nodes per core
KT = N // 128            # 32 k-tiles for the adj matmul
CI2 = CI + 1             # xh slab width: 129 + pad col

_CACHE = {}


def build_nc():
    import concourse.bass as bass
    import concourse.bacc as bacc
    import concourse.tile as tile
    import concourse.mybir as mybir

    F32 = mybir.dt.float32
    F16 = mybir.dt.float16
    ACT = mybir.ActivationFunctionType

    nc = bacc.Bacc()
    dp = nc.declare_dram_parameter
    adjT_e = dp("adjT", [128, KT * NS], F16, isOutput=False)  # [p, (kt, n)]
    xh_e = dp("xh", [128, KT * CI2], F16, isOutput=False)     # [p, (kt, f)]
    qT_e = dp("qT", [QD, NS], F16, isOutput=False)
    qrows_e = dp("qrows", [1, QD * NS], F16, isOutput=False)
    ones1_e = dp("ones1", [1, 128], F16, isOutput=False)
    x64rep_e = dp("x64rep", [QD, NS], F16, isOutput=False)    # x[:,64] tiled 16
    xTc_e = dp("xTc", [IN, NS], F16, isOutput=False)          # x[:,0:64]^T
    hT_e = dp("hT", [IN, NS], F16, isOutput=False)
    wdru_e = dp("wdru", [128, QD * 2 * IN], F16, isOutput=False)
    wdc_e = dp("wdc", [128, QD * IN], F16, isOutput=False)
    w128ru_e = dp("w128ru", [QD, 2 * IN], F16, isOutput=False)
    w128c_e = dp("w128c", [QD, IN], F16, isOutput=False)
    bru_e = dp("bru", [QD, 2 * IN], F16, isOutput=False)
    bc_e = dp("bc", [QD, IN], F16, isOutput=False)
    sel17b_e = dp("sel17b", [128, QD], F16, isOutput=False)
    out_e = dp("out", [IN, NS], F32, isOutput=True)

    with tile.TileContext(nc) as tc:
        with tc.tile_pool(name="const", bufs=1) as cpool, \
             tc.tile_pool(name="big", bufs=1) as bigpool, \
             tc.tile_pool(name="work", bufs=1) as wpool, \
             tc.tile_pool(name="zt", bufs=4) as ztpool, \
             tc.tile_pool(name="psM", bufs=1, space="PSUM") as psM, \
             tc.tile_pool(name="psQ", bufs=3, space="PSUM") as psQ:

            # ---- DMAs: tiny first (feed warmup), then xh+adjT stream -------
            qT = cpool.tile([QD, NS], F16, tag="qT")
            nc.sync.dma_start(qT[:], qT_e[:])
            qrows = cpool.tile([1, QD * NS], F16, tag="qrows")
            nc.sync.dma_start(qrows[:], qrows_e[:])
            ones1 = cpool.tile([1, 128], F16, tag="ones1")
            nc.sync.dma_start(ones1[:], ones1_e[:])
            bru = cpool.tile([QD, 2 * IN], F16, tag="bru")
            nc.sync.dma_start(bru[:], bru_e[:])
            bc = cpool.tile([QD, IN], F16, tag="bc")
            nc.sync.dma_start(bc[:], bc_e[:])
            sel17b = cpool.tile([128, QD], F16, tag="sel17b")
            nc.sync.dma_start(sel17b[:], sel17b_e[:])
            w128ru = cpool.tile([QD, 2 * IN], F16, tag="w128ru")
            nc.sync.dma_start(w128ru[:], w128ru_e[:])
            w128c = cpool.tile([QD, IN], F16, tag="w128c")
            nc.sync.dma_start(w128c[:], w128c_e[:])
            x64rep = cpool.tile([QD, NS], F16, tag="x64rep")
            nc.sync.dma_start(x64rep[:], x64rep_e[:])

            xh = bigpool.tile([128, KT * CI2], F16)
            adjT = bigpool.tile([128, KT * NS], F16)
            # xh2T rows 0:64 = h2 (written late), rows 64:128 = xT (DMA now)
            xh2T = wpool.tile([128, NS], F16, tag="xh2T")
            nc.sync.dma_start(xh2T[64:128, :], xTc_e[:])

            # interleave xh slabs with adjT slabs so matmul t never waits long
            for g in range(8):
                xlo, xhi = g * 4 * CI2, (g + 1) * 4 * CI2
                nc.sync.dma_start(xh[:, xlo:xhi], xh_e[:, xlo:xhi])
                alo, ahi = g * 4 * NS, (g + 1) * 4 * NS
                nc.sync.dma_start(adjT[:, alo:ahi], adjT_e[:, alo:ahi])
            wdru = cpool.tile([128, QD * 2 * IN], F16, tag="wdru")
            nc.sync.dma_start(wdru[:], wdru_e[:])
            wdc = cpool.tile([128, QD * IN], F16, tag="wdc")
            nc.sync.dma_start(wdc[:], wdc_e[:])
            hT = cpool.tile([IN, NS], F16, tag="hT")
            nc.sync.dma_start(hT[:], hT_e[:])

            # ---- warmup: qbc_d broadcasts + gate bias matmuls --------------
            gru = psM.tile([2 * IN, NS], F32, tag="gru")
            gc = psM.tile([IN, NS], F32, tag="gc")
            qbc = []
            for d in range(QD):
                ps = psQ.tile([128, NS], F32, tag="qb", name=f"qb{d}")
                nc.tensor.matmul(ps[:], ones1[:],
                                 qrows[0:1, d * NS:(d + 1) * NS],
                                 start=True, stop=True)
                qb = cpool.tile([128, NS], F16, tag=f"qbc{d}")
                if d % 2 == 0:
                    nc.scalar.copy(qb[:], ps[:])
                else:
                    nc.vector.tensor_copy(qb[:], ps[:])
                qbc.append(qb)
            nc.tensor.matmul(gru[:], bru[:], qT[:],
                             start=True, stop=False, skip_group_check=True)
            nc.tensor.matmul(gc[:], bc[:], qT[:],
                             start=True, stop=False, skip_group_check=True)

            # ---- adj matmul ------------------------------------------------
            pc = psM.tile([128, NS], F32, tag="pc")
            pl = psM.tile([128, NS], F32, tag="pl")
            for t in range(KT):
                rhs = adjT[:, t * NS:(t + 1) * NS]
                nc.tensor.matmul(pc[:], xh[:, t * CI2: t * CI2 + 128], rhs,
                                 start=(t == 0), stop=(t == KT - 1))
            for t in range(KT):
                rhs = adjT[:, t * NS:(t + 1) * NS]
                j = t % 4
                nc.tensor.matmul(pl[32 * j:32 * j + 1, :],
                                 xh[:, t * CI2 + 128: t * CI2 + 129], rhs,
                                 start=(t < 4), stop=(t >= KT - 4),
                                 tile_position=(0, 32 * j),
                                 skip_group_check=True)
            combT = wpool.tile([128, NS], F16, tag="combT")
            nc.scalar.copy(combT[:], pc[:])
            pl_sb = wpool.tile([128, NS], F16, tag="pl_sb")
            nc.scalar.copy(pl_sb[:], pl[:])
            v128 = psQ.tile([QD, NS], F32, tag="qb", name="v128")
            nc.tensor.matmul(v128[:], sel17b[:], pl_sb[:], start=True, stop=True)
            s_ru = wpool.tile([QD, NS], F16, tag="s_ru")
            nc.vector.tensor_mul(s_ru[:], qT[:], v128[:])

            # ---- gates r, u (d-major) --------------------------------------
            for d in range(QD):
                z = ztpool.tile([128, NS], F16, tag="z", name=f"zru{d}")
                nc.vector.tensor_mul(z[:], combT[:], qbc[d][:])
                nc.tensor.matmul(gru[:], wdru[:, d * 2 * IN:(d + 1) * 2 * IN],
                                 z[:], start=False, stop=False,
                                 skip_group_check=True)
            nc.tensor.matmul(gru[:], w128ru[:], s_ru[:],
                             start=False, stop=True, skip_group_check=True)
            r_sb = wpool.tile([IN, NS], F16, tag="r_sb")
            nc.scalar.activation(r_sb[:], gru[0:IN, :], ACT.Sigmoid)
            u_sb = wpool.tile([IN, NS], F16, tag="u_sb")
            nc.scalar.activation(u_sb[:], gru[IN:2 * IN, :], ACT.Sigmoid)

            # h2 = r*h -> xh2T rows 0:64; tail s_c = q (.) x64 (host-sent)
            nc.vector.tensor_mul(xh2T[0:IN, :], r_sb[:], hT[:])
            s_c = wpool.tile([QD, NS], F16, tag="s_c")
            nc.vector.tensor_mul(s_c[:], qT[:], x64rep[:])

            # ---- gate c (d-major over [h2 | x]) ----------------------------
            for d in range(QD):
                z = ztpool.tile([128, NS], F16, tag="z", name=f"zc{d}")
                nc.vector.tensor_mul(z[:], xh2T[:], qbc[d][:])
                nc.tensor.matmul(gc[:], wdc[:, d * IN:(d + 1) * IN],
                                 z[:], start=False, stop=False,
                                 skip_group_check=True)
            nc.tensor.matmul(gc[:], w128c[:], s_c[:],
                             start=False, stop=True, skip_group_check=True)
            cand = wpool.tile([IN, NS], F16, tag="cand")
            nc.scalar.activation(cand[:], gc[:], ACT.Tanh)

            # ---- out = h2 + u*(cand - h2) ----------------------------------
            dt_ = wpool.tile([IN, NS], F16, tag="dt")
            nc.vector.tensor_sub(dt_[:], cand[:], xh2T[0:IN, :])
            et = wpool.tile([IN, NS], F16, tag="et")
            nc.vector.tensor_mul(et[:], u_sb[:], dt_[:])
            outT = wpool.tile([IN, NS], F32, tag="outT")
            nc.vector.tensor_add(outT[:], xh2T[0:IN, :], et[:])
            nc.sync.dma_start(out_e[:], outT[:])
    nc.compile()
    return nc


def _f16(a):
    return np.ascontiguousarray(np.asarray(a, np.float16))


def prep_in_maps(x, h, query_vectors, adj, nodes_ind, W_r, b_r, W_u, b_u, W_c, b_c):
    x = np.asarray(x, np.float32)
    h = np.asarray(h, np.float32)
    q = np.asarray(query_vectors, np.float32)
    adj = np.asarray(adj, np.float32)
    ni = np.asarray(nodes_ind)
    assert np.array_equal(ni, np.arange(N)), "kernel assumes nodes_ind == arange(N)"

    xh = np.concatenate([x, h, np.zeros((N, 1), np.float32)], axis=-1)  # [N,130]
    xh_sb = _f16(xh.reshape(KT, 128, CI2).transpose(1, 0, 2).reshape(128, KT * CI2))

    # d-major weight blocks. ru: block d = [W_r[d, i<128] | W_u[d, i<128]],
    # tail (i=128) separate. c: feature order [h2 (orig i 65..128) | x (0..63)],
    # tail = orig i 64 (x[:,64]).
    Wr = np.asarray(W_r, np.float32)
    Wu = np.asarray(W_u, np.float32)
    Wc = np.asarray(W_c, np.float32)
    wdru = np.concatenate([Wr[:, :128, :], Wu[:, :128, :]], axis=2)  # [16,128,128]
    wdru = _f16(wdru.transpose(1, 0, 2).reshape(128, QD * 2 * IN))
    perm_c = list(range(65, CI)) + list(range(0, 64))                # [h2|x]
    wdc = Wc[:, perm_c, :]                                           # [16,128,64]
    wdc = _f16(wdc.transpose(1, 0, 2).reshape(128, QD * IN))
    w128ru = _f16(np.concatenate([Wr[:, 128, :], Wu[:, 128, :]], axis=1))
    w128c = _f16(Wc[:, 64, :])
    bru = _f16(np.concatenate([np.asarray(b_r, np.float32),
                               np.asarray(b_u, np.float32)], axis=1))
    bc = _f16(np.asarray(b_c, np.float32))

    sel17b = np.zeros((128, QD), np.float32)
    for jj in range(4):
        sel17b[32 * jj, :] = 1.0
    ones1 = np.ones((1, 128), np.float32)

    in_maps = []
    for c in range(NCORES):
        s = slice(c * NS, (c + 1) * NS)
        adjT_sb = _f16(adj[s].T.reshape(KT, 128, NS).transpose(1, 0, 2)
                       .reshape(128, KT * NS))
        in_maps.append({
            "adjT": adjT_sb,
            "xh": xh_sb,
            "qT": _f16(q[s].T),
            "qrows": _f16(q[s].T.reshape(1, QD * NS)),
            "ones1": _f16(ones1),
            "x64rep": _f16(np.tile(x[s, 64], (QD, 1))),
            "xTc": _f16(x[s, 0:64].T),
            "hT": _f16(h[s].T),
            "wdru": wdru, "wdc": wdc,
            "w128ru": w128ru, "w128c": w128c,
            "bru": bru, "bc": bc,
            "sel17b": _f16(sel17b),
        })
    return in_maps


def kernel(**inputs):
    from concourse.bass_utils import run_bass_kernel_spmd

    if "nc" not in _CACHE:
        _CACHE["nc"] = build_nc()
    nc = _CACHE["nc"]
    in_maps = prep_in_maps(**inputs)
    res = run_bass_kernel_spmd(nc, in_maps, core_ids=list(range(NCORES)))
    out = np.empty((N, IN), np.float32)
    for c in range(NCORES):
        out[c * NS:(c + 1) * NS, :] = res.results[c]["out"].T
    return out
